# revision 11
# baseline (speedup 1.0000x reference)
"""Differential attention Trainium2 kernel (Bass/Tile), 8-core data parallel.

Sharding: core c handles batch b = c//2 and query half h = c%2.
Each core receives x[b]^T (bf16, host-transposed; key order rolled by 2048
for odd cores so "my queries" are always columns 0:2048).

On-chip pipeline (all hot paths inside hardware For_i loops — this runtime
charges a superlinear per-execution cost on straight-line instruction count,
so static program size is kept tiny):
  L1a: K^T / Q^T projections per 512-token chunk
  L1b: V^T projection
  L2:  V (natural [k, e]) via PE transpose of V^T
  L3:  per 128-query tile: A1/A2 logits (row-packed K=64 matmuls) -> exp on
       ScalarE (scale=1/8 folded, accum_out = row sums) -> combined via 2 DVE
       ops (per-partition scalars) -> DRAM (SWDGE bf16->fp32 cast DMA) ->
       combined^T via xbar DMA transpose -> AV matmul (inner For_i, 32
       accumulating MMs) -> output rows.
Softmax max-subtraction is skipped: logits ~N(0, 0.33); exp cannot overflow
and softmax(x) == softmax(x - max) up to fp rounding.
"""

import math
import sys

sys.path.insert(0, "/opt/trn_rl_repo")

import ml_dtypes
import numpy as np

B, S, D = 4, 4096, 1024
HD = 64
E = 2 * HD  # 128
P = 128
DEPTH = 12
NCORES = 8

_BUILD_CACHE: dict = {}


def _build(lam: float, s: int, d: int, qh: int, n_iter: int = 1, av_loop: bool = True):
    from contextlib import ExitStack

    import concourse.mybir as mybir
    import concourse.tile as tile
    from concourse import bacc
    from concourse.bass import ds, ts
    from concourse.masks import make_identity

    F32 = mybir.dt.float32
    BF16 = mybir.dt.bfloat16
    AF = mybir.ActivationFunctionType
    OP = mybir.AluOpType

    DC = d // P            # d-chunks (8)
    NKT = s // P           # 128-wide key tiles (32)
    NQT = qh // P          # 128-query tiles (16)
    KHALF = s // 2         # 2048
    ACW = min(512, KHALF)  # A-matmul chunk width
    NKC = KHALF // ACW     # chunks per half
    PCW = min(512, s)      # projection chunk width
    NSC = s // PCW         # projection chunks (8)
    scale = HD ** -0.5

    nc = bacc.Bacc(
        "TRN2",
        target_bir_lowering=False,
        debug=False,
        enable_asserts=False,
        num_devices=1,
    )
    xt_d = nc.dram_tensor("xt", [d, s], BF16, kind="ExternalInput")
    wq_d = nc.dram_tensor("wqt", [d, E], BF16, kind="ExternalInput")
    wk_d = nc.dram_tensor("wkt", [d, E], BF16, kind="ExternalInput")
    wv_d = nc.dram_tensor("wvt", [d, E], BF16, kind="ExternalInput")
    comb_d = nc.dram_tensor("comb", [qh, s], F32, kind="ExternalOutput")
    out_d = nc.dram_tensor("out", [qh, E], F32, kind="ExternalOutput")

    with tile.TileContext(nc) as tc, ExitStack() as ctx:
        const = ctx.enter_context(tc.tile_pool(name="const", bufs=1))
        work = ctx.enter_context(tc.tile_pool(name="work", bufs=1))
        psum = ctx.enter_context(tc.tile_pool(name="psum", bufs=2, space="PSUM"))

        wq = const.tile([P, DC, E], BF16, tag="wq")
        wk = const.tile([P, DC, E], BF16, tag="wk")
        wv = const.tile([P, DC, E], BF16, tag="wv")
        nc.sync.dma_start(wq[:], wq_d.ap().rearrange("(c p) e -> p c e", p=P))
        nc.sync.dma_start(wk[:], wk_d.ap().rearrange("(c p) e -> p c e", p=P))
        nc.sync.dma_start(wv[:], wv_d.ap().rearrange("(c p) e -> p c e", p=P))
        ident = const.tile([P, P], BF16, tag="ident")
        make_identity(nc, ident[:])

        KT = const.tile([P, s], BF16, tag="KT")    # [e, k]
        QTf = const.tile([P, s], BF16, tag="QTf")  # [e, s] (cols 0:qh used)
        VT = const.tile([P, s], BF16, tag="VT")    # [e, k]
        V = const.tile([P, NKT * E], BF16, tag="V")  # [:, kt*128:+128] = [k, e]

        xt_re = xt_d.ap().rearrange("(c p) s -> p c s", p=P)

        for _it in range(n_iter):
            # L1a: K^T and Q^T per s-chunk
            with tc.For_i(0, NSC, 1) as iv:
                xt_c = work.tile([P, DC, PCW], BF16, tag="xtc")
                nc.sync.dma_start(xt_c[:], xt_re[:, :, ds(iv * PCW, PCW)])
                psK = psum.tile([P, PCW], F32, tag="ps")
                psQ = psum.tile([P, PCW], F32, tag="ps")
                for dc in range(DC):
                    nc.tensor.matmul(psK[:], wk[:, dc, :], xt_c[:, dc, :],
                                     start=(dc == 0), stop=(dc == DC - 1))
                for dc in range(DC):
                    nc.tensor.matmul(psQ[:], wq[:, dc, :], xt_c[:, dc, :],
                                     start=(dc == 0), stop=(dc == DC - 1))
                nc.vector.tensor_copy(KT[:, ds(iv * PCW, PCW)], psK[:])
                nc.vector.tensor_copy(QTf[:, ds(iv * PCW, PCW)], psQ[:])

            # L1b: V^T per s-chunk
            with tc.For_i(0, NSC, 1) as iv:
                xt_c2 = work.tile([P, DC, PCW], BF16, tag="xtc2")
                nc.sync.dma_start(xt_c2[:], xt_re[:, :, ds(iv * PCW, PCW)])
                psV = psum.tile([P, PCW], F32, tag="ps")
                for dc in range(DC):
                    nc.tensor.matmul(psV[:], wv[:, dc, :], xt_c2[:, dc, :],
                                     start=(dc == 0), stop=(dc == DC - 1))
                nc.vector.tensor_copy(VT[:, ds(iv * PCW, PCW)], psV[:])

            # L2: V[k, e] tiles via PE transpose
            with tc.For_i(0, NKT, 1) as iv:
                vsl = work.tile([P, P], BF16, tag="vsl")
                nc.vector.tensor_copy(vsl[:], VT[:, ds(iv * P, P)])
                psT = psum.tile([P, P], BF16, tag="ps")
                nc.tensor.transpose(psT[:], vsl[:], ident[:])
                nc.vector.tensor_copy(V[:, ds(iv * P, P)], psT[:])

            # L3: attention per q-tile
            with tc.For_i(0, NQT, 1) as iv:
                qsl = work.tile([P, P], BF16, tag="qsl")
                nc.vector.tensor_copy(qsl[:], QTf[:, ds(iv * P, P)])
                exp1 = work.tile([P, s], BF16, tag="exp1")
                exp2 = work.tile([P, s], BF16, tag="exp2")
                sp = work.tile([P, 4], F32, tag="sp")
                for half in range(2):
                    hb = half * KHALF
                    psA1 = psum.tile([P, KHALF], F32, tag="ps")
                    psA2 = psum.tile([P, KHALF], F32, tag="ps")
                    if NKC > 1:
                        with tc.For_i(0, NKC, 1) as kc:
                            nc.tensor.matmul(
                                psA1[:, ds(kc * ACW, ACW)], qsl[0:64, :],
                                KT[0:64, ds(hb + kc * ACW, ACW)],
                                start=True, stop=True)
                            nc.tensor.matmul(
                                psA2[:, ds(kc * ACW, ACW)], qsl[64:128, :],
                                KT[64:128, ds(hb + kc * ACW, ACW)],
                                start=True, stop=True, tile_position=(64, 0))
                    else:
                        nc.tensor.matmul(psA1[:], qsl[0:64, :],
                                         KT[0:64, ds(hb, ACW)],
                                         start=True, stop=True)
                        nc.tensor.matmul(psA2[:], qsl[64:128, :],
                                         KT[64:128, ds(hb, ACW)],
                                         start=True, stop=True,
                                         tile_position=(64, 0))
                    nc.scalar.activation(
                        exp1[:, ds(hb, KHALF)], psA1[:], AF.Exp,
                        scale=scale, accum_out=sp[:, 0 + half : 1 + half])
                    nc.scalar.activation(
                        exp2[:, ds(hb, KHALF)], psA2[:], AF.Exp,
                        scale=scale, accum_out=sp[:, 2 + half : 3 + half])
                s12 = work.tile([P, 2], F32, tag="s12")
                nc.vector.tensor_tensor(s12[:, 0:1], sp[:, 0:1], sp[:, 1:2], OP.add)
                nc.vector.tensor_tensor(s12[:, 1:2], sp[:, 2:3], sp[:, 3:4], OP.add)
                r12 = work.tile([P, 2], F32, tag="r12")
                nc.vector.reciprocal(r12[:], s12[:])

                tmp = work.tile([P, s], BF16, tag="tmp")
                nc.vector.tensor_scalar(tmp[:], exp2[:], r12[:, 1:2], lam,
                                        OP.mult, OP.mult)
                comb = work.tile([P, s], BF16, tag="comb")
                nc.vector.scalar_tensor_tensor(
                    comb[:], exp1[:], r12[:, 0:1], tmp[:], OP.mult, OP.subtract)

                nc.gpsimd.dma_start(comb_d.ap()[ds(iv * P, P), :], comb[:])

                cT = work.tile([P, NKT, P], BF16, tag="cT")
                nc.sync.dma_start_transpose(cT[:], comb[:])
                psO = psum.tile([P, E], F32, tag="ps")
                cT2 = cT[:].rearrange("p c f -> p (c f)")
                if av_loop:
                    nc.vector.memset(psO[:], 0)
                    with tc.For_i(0, NKT, 1) as c:
                        csl = work.tile([P, P], BF16, tag="csl")
                        nc.vector.tensor_copy(csl[:], cT2[:, ds(c * P, P)])
                        nc.tensor.matmul(psO[:], csl[:],
                                         V[:, ds(c * P, P)],
                                         start=False, stop=False,
                                         skip_group_check=True)
                else:
                    for c in range(NKT):
                        nc.tensor.matmul(psO[:], cT[:, c, :],
                                         V[:, ds(c * P, P)],
                                         start=(c == 0), stop=(c == NKT - 1))
                ob = work.tile([P, E], F32, tag="ob")
                nc.vector.tensor_copy(ob[:], psO[:])
                nc.sync.dma_start(out_d.ap()[ds(iv * P, P), :], ob[:])

    nc.compile()
    return nc


def _get_nc(lam: float, n_iter: int = 1):
    key = (round(lam, 6), n_iter)
    if key not in _BUILD_CACHE:
        _BUILD_CACHE[key] = _build(lam, S, D, S // 2, n_iter)
    return _BUILD_CACHE[key]


def _prep_inputs(x, Wq, Wk, Wv):
    bf = ml_dtypes.bfloat16
    x = np.asarray(x, dtype=np.float32)
    wqt = np.ascontiguousarray(np.asarray(Wq, np.float32).T).astype(bf)
    wkt = np.ascontiguousarray(np.asarray(Wk, np.float32).T).astype(bf)
    wvt = np.ascontiguousarray(np.asarray(Wv, np.float32).T).astype(bf)
    roll = np.r_[S // 2 : S, 0 : S // 2]
    in_maps = []
    for c in range(NCORES):
        b, h = divmod(c, 2)
        xb = x[b] if h == 0 else x[b][roll]
        xt = np.ascontiguousarray(xb.T).astype(bf)
        in_maps.append({"xt": xt, "wqt": wqt, "wkt": wkt, "wvt": wvt})
    return in_maps, roll


def kernel(x, Wq, Wk, Wv, lambda_q1, lambda_q2, lambda_k1, lambda_k2):
    from concourse.bass_utils import run_bass_kernel_spmd

    lq1 = np.asarray(lambda_q1, np.float64)
    lq2 = np.asarray(lambda_q2, np.float64)
    lk1 = np.asarray(lambda_k1, np.float64)
    lk2 = np.asarray(lambda_k2, np.float64)
    lam_init = 0.8 - 0.6 * math.exp(-0.3 * DEPTH)
    lam = float(
        np.exp(np.sum(lq1 * lk1)) - np.exp(np.sum(lq2 * lk2)) + lam_init
    )

    nc = _get_nc(lam)
    in_maps, roll = _prep_inputs(x, Wq, Wk, Wv)
    res = run_bass_kernel_spmd(nc, in_maps, core_ids=list(range(NCORES)))

    QH = S // 2
    combined = np.empty((B, S, S), np.float32)
    output = np.empty((B, S, E), np.float32)
    for c in range(NCORES):
        b, h = divmod(c, 2)
        comb = res.results[c]["comb"]
        out = res.results[c]["out"]
        if h == 1:
            comb = comb[:, roll]
        combined[b, h * QH : (h + 1) * QH] = comb
        output[b, h * QH : (h + 1) * QH] = out
    return output, combined


# revision 13
# speedup vs baseline: 3.4877x; 3.4877x over previous
"""Differential attention Trainium2 kernel (Bass/Tile), 8-core data parallel.

Sharding: core c handles batch b = c//2 and query half h = c%2.
Each core receives x[b]^T (bf16, host-transposed; key order rolled by 2048
for odd cores so "my queries" are always columns 0:2048).

On-chip pipeline (all hot paths inside hardware For_i loops — this runtime
charges a superlinear per-execution cost on straight-line instruction count,
so static program size is kept tiny):
  L1a: K^T / Q^T projections per 512-token chunk
  L1b: V^T projection
  L2:  V (natural [k, e]) via PE transpose of V^T
  L3:  per 128-query tile: A1/A2 logits (row-packed K=64 matmuls) -> exp on
       ScalarE (scale=1/8 folded, accum_out = row sums) -> combined via 2 DVE
       ops (per-partition scalars) -> DRAM (SWDGE bf16->fp32 cast DMA) ->
       combined^T via xbar DMA transpose -> AV matmul (inner For_i, 32
       accumulating MMs) -> output rows.
Softmax max-subtraction is skipped: logits ~N(0, 0.33); exp cannot overflow
and softmax(x) == softmax(x - max) up to fp rounding.
"""

import math
import sys

sys.path.insert(0, "/opt/trn_rl_repo")

import ml_dtypes
import numpy as np

B, S, D = 4, 4096, 1024
HD = 64
E = 2 * HD  # 128
P = 128
DEPTH = 12
NCORES = 8

_BUILD_CACHE: dict = {}


def _build(lam: float, s: int, d: int, qh: int, n_iter: int = 1):
    from contextlib import ExitStack

    import concourse.mybir as mybir
    import concourse.tile as tile
    from concourse import bacc
    from concourse.bass import ds, ts
    from concourse.masks import make_identity

    F32 = mybir.dt.float32
    BF16 = mybir.dt.bfloat16
    AF = mybir.ActivationFunctionType
    OP = mybir.AluOpType

    DC = d // P            # d-chunks (8)
    NKT = s // P           # 128-wide key tiles (32)
    NQT = qh // P          # 128-query tiles (16)
    KHALF = s // 2         # 2048
    ACW = min(512, KHALF)  # A-matmul chunk width
    NKC = KHALF // ACW     # chunks per half
    PCW = min(512, s)      # projection chunk width
    NSC = s // PCW         # projection chunks (8)
    scale = HD ** -0.5

    nc = bacc.Bacc(
        "TRN2",
        target_bir_lowering=False,
        debug=False,
        enable_asserts=False,
        num_devices=1,
    )
    xt_d = nc.dram_tensor("xt", [d, s], BF16, kind="ExternalInput")
    wq_d = nc.dram_tensor("wqt", [d, E], BF16, kind="ExternalInput")
    wk_d = nc.dram_tensor("wkt", [d, E], BF16, kind="ExternalInput")
    wv_d = nc.dram_tensor("wvt", [d, E], BF16, kind="ExternalInput")
    comb_d = nc.dram_tensor("comb", [qh, s], F32, kind="ExternalOutput")
    out_d = nc.dram_tensor("out", [E, qh], F32, kind="ExternalOutput")

    with tile.TileContext(nc) as tc, ExitStack() as ctx:
        const = ctx.enter_context(tc.tile_pool(name="const", bufs=1))
        work = ctx.enter_context(tc.tile_pool(name="work", bufs=1))
        psum = ctx.enter_context(tc.tile_pool(name="psum", bufs=2, space="PSUM"))
        dram = ctx.enter_context(tc.tile_pool(name="dram", bufs=1, space="DRAM"))
        combbf_d = dram.tile([qh, s], BF16, tag="combbf")

        wq = const.tile([P, DC, E], BF16, tag="wq")
        wk = const.tile([P, DC, E], BF16, tag="wk")
        wv = const.tile([P, DC, E], BF16, tag="wv")
        nc.sync.dma_start(wq[:], wq_d.ap().rearrange("(c p) e -> p c e", p=P))
        nc.sync.dma_start(wk[:], wk_d.ap().rearrange("(c p) e -> p c e", p=P))
        nc.sync.dma_start(wv[:], wv_d.ap().rearrange("(c p) e -> p c e", p=P))
        ident = const.tile([P, P], BF16, tag="ident")
        make_identity(nc, ident[:])

        KT = const.tile([P, s], BF16, tag="KT")    # [e, k]
        QTf = const.tile([P, s], BF16, tag="QTf")  # [e, s] (cols 0:qh used)
        VT = const.tile([P, s], BF16, tag="VT")    # [e, k]
        V = const.tile([P, NKT * E], BF16, tag="V")  # [:, kt*128:+128] = [k, e]

        xt_re = xt_d.ap().rearrange("(c p) s -> p c s", p=P)

        for _it in range(n_iter):
            # L1a: K^T and Q^T per s-chunk
            with tc.For_i(0, NSC, 1) as iv:
                xt_c = work.tile([P, DC, PCW], BF16, tag="xtc")
                nc.sync.dma_start(xt_c[:], xt_re[:, :, ds(iv * PCW, PCW)])
                psK = psum.tile([P, PCW], F32, tag="ps")
                psQ = psum.tile([P, PCW], F32, tag="ps")
                for dc in range(DC):
                    nc.tensor.matmul(psK[:], wk[:, dc, :], xt_c[:, dc, :],
                                     start=(dc == 0), stop=(dc == DC - 1))
                for dc in range(DC):
                    nc.tensor.matmul(psQ[:], wq[:, dc, :], xt_c[:, dc, :],
                                     start=(dc == 0), stop=(dc == DC - 1))
                nc.vector.tensor_copy(KT[:, ds(iv * PCW, PCW)], psK[:])
                nc.vector.tensor_copy(QTf[:, ds(iv * PCW, PCW)], psQ[:])

            # L1b: V^T per s-chunk
            with tc.For_i(0, NSC, 1) as iv:
                xt_c2 = work.tile([P, DC, PCW], BF16, tag="xtc2")
                nc.sync.dma_start(xt_c2[:], xt_re[:, :, ds(iv * PCW, PCW)])
                psV = psum.tile([P, PCW], F32, tag="ps")
                for dc in range(DC):
                    nc.tensor.matmul(psV[:], wv[:, dc, :], xt_c2[:, dc, :],
                                     start=(dc == 0), stop=(dc == DC - 1))
                nc.vector.tensor_copy(VT[:, ds(iv * PCW, PCW)], psV[:])

            # L2: V[k, e] tiles via PE transpose
            with tc.For_i(0, NKT, 1) as iv:
                vsl = work.tile([P, P], BF16, tag="vsl")
                nc.vector.tensor_copy(vsl[:], VT[:, ds(iv * P, P)])
                psT = psum.tile([P, P], BF16, tag="ps")
                nc.tensor.transpose(psT[:], vsl[:], ident[:])
                nc.vector.tensor_copy(V[:, ds(iv * P, P)], psT[:])

            # LoopA: per q-tile: logits -> exp -> combined -> DRAM (fp32 + bf16)
            with tc.For_i(0, NQT, 1) as iv:
                qsl = work.tile([P, P], BF16, tag="qsl")
                nc.vector.tensor_copy(qsl[:], QTf[:, ds(iv * P, P)])
                exp1 = work.tile([P, s], BF16, tag="exp1")
                exp2 = work.tile([P, s], BF16, tag="exp2")
                sp = work.tile([P, 4], F32, tag="sp")
                for half in range(2):
                    hb = half * KHALF
                    psA1 = psum.tile([P, KHALF], F32, tag="ps")
                    psA2 = psum.tile([P, KHALF], F32, tag="ps")
                    for kc in range(NKC):
                        nc.tensor.matmul(
                            psA1[:, ts(kc, ACW)], qsl[0:64, :],
                            KT[0:64, ds(hb + kc * ACW, ACW)],
                            start=True, stop=True)
                        nc.tensor.matmul(
                            psA2[:, ts(kc, ACW)], qsl[64:128, :],
                            KT[64:128, ds(hb + kc * ACW, ACW)],
                            start=True, stop=True, tile_position=(64, 0))
                    nc.scalar.activation(
                        exp1[:, ds(hb, KHALF)], psA1[:], AF.Exp,
                        scale=scale, accum_out=sp[:, 0 + half : 1 + half])
                    nc.scalar.activation(
                        exp2[:, ds(hb, KHALF)], psA2[:], AF.Exp,
                        scale=scale, accum_out=sp[:, 2 + half : 3 + half])
                s12 = work.tile([P, 2], F32, tag="s12")
                nc.vector.tensor_tensor(s12[:, 0:1], sp[:, 0:1], sp[:, 1:2], OP.add)
                nc.vector.tensor_tensor(s12[:, 1:2], sp[:, 2:3], sp[:, 3:4], OP.add)
                r12 = work.tile([P, 2], F32, tag="r12")
                nc.vector.reciprocal(r12[:], s12[:])

                tmp = work.tile([P, s], BF16, tag="tmp")
                nc.vector.tensor_scalar(tmp[:], exp2[:], r12[:, 1:2], lam,
                                        OP.mult, OP.mult)
                comb = work.tile([P, s], BF16, tag="comb")
                nc.vector.scalar_tensor_tensor(
                    comb[:], exp1[:], r12[:, 0:1], tmp[:], OP.mult, OP.subtract)

                nc.gpsimd.dma_start(comb_d.ap()[ds(iv * P, P), :], comb[:])
                nc.sync.dma_start(combbf_d[ds(iv * P, P), :], comb[:])

            # LoopB: per 512-query block: combined^T via xbar from DRAM,
            # then AV with N=512 (static stationary operands)
            BW = min(512, qh)
            NBLK = qh // BW
            outT = const.tile([P, qh], F32, tag="outT")  # out^T [e, q]
            with tc.For_i(0, NBLK, 1) as bv:
                cT4 = work.tile([P, NKT, BW], BF16, tag="cT4")
                nc.sync.dma_start_transpose(
                    cT4[:], combbf_d[ds(bv * BW, BW), :])
                psOT = psum.tile([P, BW], F32, tag="ps")
                for c in range(NKT):
                    nc.tensor.matmul(psOT[:], V[:, ts(c, P)], cT4[:, c, :],
                                     start=(c == 0), stop=(c == NKT - 1))
                nc.vector.tensor_copy(outT[:, ds(bv * BW, BW)], psOT[:])
            nc.sync.dma_start(out_d.ap(), outT[:])

    nc.compile()
    return nc


def _get_nc(lam: float, n_iter: int = 1):
    key = (round(lam, 6), n_iter)
    if key not in _BUILD_CACHE:
        _BUILD_CACHE[key] = _build(lam, S, D, S // 2, n_iter)
    return _BUILD_CACHE[key]


def _prep_inputs(x, Wq, Wk, Wv):
    bf = ml_dtypes.bfloat16
    x = np.asarray(x, dtype=np.float32)
    wqt = np.ascontiguousarray(np.asarray(Wq, np.float32).T).astype(bf)
    wkt = np.ascontiguousarray(np.asarray(Wk, np.float32).T).astype(bf)
    wvt = np.ascontiguousarray(np.asarray(Wv, np.float32).T).astype(bf)
    roll = np.r_[S // 2 : S, 0 : S // 2]
    in_maps = []
    for c in range(NCORES):
        b, h = divmod(c, 2)
        xb = x[b] if h == 0 else x[b][roll]
        xt = np.ascontiguousarray(xb.T).astype(bf)
        in_maps.append({"xt": xt, "wqt": wqt, "wkt": wkt, "wvt": wvt})
    return in_maps, roll


def kernel(x, Wq, Wk, Wv, lambda_q1, lambda_q2, lambda_k1, lambda_k2):
    from concourse.bass_utils import run_bass_kernel_spmd

    lq1 = np.asarray(lambda_q1, np.float64)
    lq2 = np.asarray(lambda_q2, np.float64)
    lk1 = np.asarray(lambda_k1, np.float64)
    lk2 = np.asarray(lambda_k2, np.float64)
    lam_init = 0.8 - 0.6 * math.exp(-0.3 * DEPTH)
    lam = float(
        np.exp(np.sum(lq1 * lk1)) - np.exp(np.sum(lq2 * lk2)) + lam_init
    )

    nc = _get_nc(lam)
    in_maps, roll = _prep_inputs(x, Wq, Wk, Wv)
    res = run_bass_kernel_spmd(nc, in_maps, core_ids=list(range(NCORES)))

    QH = S // 2
    combined = np.empty((B, S, S), np.float32)
    output = np.empty((B, S, E), np.float32)
    for c in range(NCORES):
        b, h = divmod(c, 2)
        comb = res.results[c]["comb"]
        out = res.results[c]["out"]
        if h == 1:
            comb = comb[:, roll]
        combined[b, h * QH : (h + 1) * QH] = comb
        output[b, h * QH : (h + 1) * QH] = out.T
    return output, combined


# revision 16
# speedup vs baseline: 5.4866x; 1.5731x over previous
"""Differential attention Trainium2 kernel (Bass/Tile), 8-core data parallel.

Sharding: core c handles batch b = c//2 and query half h = c%2.
Each core receives x[b]^T (bf16, host-transposed; key order rolled by 2048
for odd cores so "my queries" are always columns 0:2048).

This runtime charges a large per-execution cost both for straight-line
instruction count and for register-offset (dynamic) access patterns on
compute engines, so the kernel is structured as hardware For_i loops whose
bodies use only STATIC compute APs; all per-iteration variation flows
through dynamic-DRAM-offset DMAs (cheap) and fixed SBUF tiles:
  L1a: K^T and Q^T projections per 512-token chunk -> DRAM scratch
  L1b: V^T projection -> DRAM scratch
  L2:  V (natural [k, e]) via PE transpose -> DRAM scratch
  LoopA: per 128-query tile: A1/A2 logits (row-packed K=64 matmuls) -> exp
       on ScalarE (scale=1/8 folded, accum_out = row sums) -> combined via
       2 DVE ops (per-partition scalars) -> DRAM fp32 (SWDGE cast DMA) and
       DRAM bf16 scratch
  LoopB: per 512-query block: combined^T via xbar DMA transpose from DRAM
       -> AV matmul (32 static N=512 MMs) -> out^T -> DRAM
Softmax max-subtraction is skipped: logits ~N(0, 0.33); exp cannot overflow
and softmax(x) == softmax(x - max) up to fp rounding.
"""

import math
import sys

sys.path.insert(0, "/opt/trn_rl_repo")

import ml_dtypes
import numpy as np

B, S, D = 4, 4096, 1024
HD = 64
E = 2 * HD  # 128
P = 128
DEPTH = 12
NCORES = 8

_BUILD_CACHE: dict = {}


def _build(lam: float, s: int, d: int, qh: int, n_iter: int = 1,
           timing: bool = False, stages: str = "l1a,l1b,l2,la,lb"):
    from contextlib import ExitStack

    import concourse.mybir as mybir
    import concourse.tile as tile
    from concourse import bacc
    from concourse.bass import ds, ts
    from concourse.masks import make_identity

    F32 = mybir.dt.float32
    BF16 = mybir.dt.bfloat16
    AF = mybir.ActivationFunctionType
    OP = mybir.AluOpType

    DC = d // P            # d-chunks (8)
    NKT = s // P           # 128-wide key tiles (32)
    NQT = qh // P          # 128-query tiles (16)
    KHALF = s // 2         # 2048
    ACW = min(512, KHALF)  # A-matmul chunk width
    NKC = KHALF // ACW     # chunks per half
    PCW = min(512, s)      # projection chunk width
    NSC = s // PCW         # projection chunks (8)
    BW = min(512, qh)      # AV block width
    NBLK = qh // BW
    scale = HD ** -0.5
    stage_set = set(stages.split(","))

    nc = bacc.Bacc(
        "TRN2",
        target_bir_lowering=False,
        debug=False,
        enable_asserts=False,
        num_devices=1,
    )
    xt_d = nc.dram_tensor("xt", [d, s], BF16, kind="ExternalInput")
    wq_d = nc.dram_tensor("wqt", [d, E], BF16, kind="ExternalInput")
    wk_d = nc.dram_tensor("wkt", [d, E], BF16, kind="ExternalInput")
    wv_d = nc.dram_tensor("wvt", [d, E], BF16, kind="ExternalInput")
    if timing:
        comb_d = nc.dram_tensor("comb", [qh, s], F32, kind="Internal")
    else:
        comb_d = nc.dram_tensor("comb", [qh, s], F32, kind="ExternalOutput")
    out_d = nc.dram_tensor("out", [E, qh], F32, kind="ExternalOutput")

    with tile.TileContext(nc) as tc, ExitStack() as ctx:
        const = ctx.enter_context(tc.tile_pool(name="const", bufs=1))
        work = ctx.enter_context(tc.tile_pool(name="work", bufs=1))
        psum = ctx.enter_context(tc.tile_pool(name="psum", bufs=2, space="PSUM"))
        dram = ctx.enter_context(tc.tile_pool(name="dram", bufs=1, space="DRAM"))

        combbf_d = dram.tile([qh, s], BF16, tag="combbf")
        kt_dram = dram.tile([P, s], BF16, tag="ktd")
        qt_dram = dram.tile([P, s], BF16, tag="qtd")
        vt_dram = dram.tile([P, s], BF16, tag="vtd")
        v_dram = dram.tile([s, E], BF16, tag="vd")

        wq = const.tile([P, DC, E], BF16, tag="wq")
        wk = const.tile([P, DC, E], BF16, tag="wk")
        wv = const.tile([P, DC, E], BF16, tag="wv")
        nc.sync.dma_start(wq[:], wq_d.ap().rearrange("(c p) e -> p c e", p=P))
        nc.sync.dma_start(wk[:], wk_d.ap().rearrange("(c p) e -> p c e", p=P))
        nc.sync.dma_start(wv[:], wv_d.ap().rearrange("(c p) e -> p c e", p=P))
        ident = const.tile([P, P], BF16, tag="ident")
        make_identity(nc, ident[:])

        KT = const.tile([P, s], BF16, tag="KT")      # [e, k]
        V = const.tile([P, NKT, E], BF16, tag="V")  # [:, c, :] = [k, e]

        xt_re = xt_d.ap().rearrange("(c p) s -> p c s", p=P)

        for _it in range(n_iter):
            # L1a: K^T and Q^T per s-chunk -> DRAM scratch
            if "l1a" in stage_set:
                with tc.For_i(0, NSC, 1) as iv:
                    xt_c = work.tile([P, DC, PCW], BF16, tag="xtc")
                    nc.sync.dma_start(xt_c[:], xt_re[:, :, ds(iv * PCW, PCW)])
                    psK = psum.tile([P, PCW], F32, tag="ps")
                    psQ = psum.tile([P, PCW], F32, tag="ps")
                    for dc in range(DC):
                        nc.tensor.matmul(psK[:], wk[:, dc, :], xt_c[:, dc, :],
                                         start=(dc == 0), stop=(dc == DC - 1))
                    for dc in range(DC):
                        nc.tensor.matmul(psQ[:], wq[:, dc, :], xt_c[:, dc, :],
                                         start=(dc == 0), stop=(dc == DC - 1))
                    kc_sb = work.tile([P, PCW], BF16, tag="kc_sb")
                    qc_sb = work.tile([P, PCW], BF16, tag="qc_sb")
                    nc.vector.tensor_copy(kc_sb[:], psK[:])
                    nc.vector.tensor_copy(qc_sb[:], psQ[:])
                    nc.sync.dma_start(kt_dram[:, ds(iv * PCW, PCW)], kc_sb[:])
                    nc.sync.dma_start(qt_dram[:, ds(iv * PCW, PCW)], qc_sb[:])

            # L1b: V^T per s-chunk -> DRAM scratch
            if "l1b" in stage_set:
                with tc.For_i(0, NSC, 1) as iv:
                    xt_c2 = work.tile([P, DC, PCW], BF16, tag="xtc2")
                    nc.sync.dma_start(xt_c2[:], xt_re[:, :, ds(iv * PCW, PCW)])
                    psV = psum.tile([P, PCW], F32, tag="ps")
                    for dc in range(DC):
                        nc.tensor.matmul(psV[:], wv[:, dc, :], xt_c2[:, dc, :],
                                         start=(dc == 0), stop=(dc == DC - 1))
                    vc_sb = work.tile([P, PCW], BF16, tag="vc_sb")
                    nc.vector.tensor_copy(vc_sb[:], psV[:])
                    nc.sync.dma_start(vt_dram[:, ds(iv * PCW, PCW)], vc_sb[:])

            # L2: V[k, e] via PE transpose -> DRAM scratch
            if "l2" in stage_set:
                with tc.For_i(0, NKT, 1) as iv:
                    vsl = work.tile([P, P], BF16, tag="vsl")
                    nc.sync.dma_start(vsl[:], vt_dram[:, ds(iv * P, P)])
                    psT = psum.tile([P, P], BF16, tag="ps")
                    nc.tensor.transpose(psT[:], vsl[:], ident[:])
                    vv_sb = work.tile([P, P], BF16, tag="vv_sb")
                    nc.vector.tensor_copy(vv_sb[:], psT[:])
                    nc.sync.dma_start(v_dram[ds(iv * P, P), :], vv_sb[:])

            # gather scratch -> SBUF (static)
            nc.sync.dma_start(KT[:], kt_dram[:, :])
            nc.sync.dma_start(V[:], v_dram[:, :].rearrange("(c p) e -> p c e", p=P))

            # LoopA: per q-tile: logits -> exp -> combined -> DRAM
            if "la" in stage_set:
                with tc.For_i(0, NQT, 1) as iv:
                    qsl = work.tile([P, P], BF16, tag="qsl")
                    nc.sync.dma_start(qsl[:], qt_dram[:, ds(iv * P, P)])
                    exp1 = work.tile([P, s], BF16, tag="exp1")
                    exp2 = work.tile([P, s], BF16, tag="exp2")
                    sp = work.tile([P, 4], F32, tag="sp")
                    for half in range(2):
                        hb = half * KHALF
                        psA1 = psum.tile([P, KHALF], F32, tag="ps")
                        psA2 = psum.tile([P, KHALF], F32, tag="ps")
                        for kc in range(NKC):
                            nc.tensor.matmul(
                                psA1[:, ts(kc, ACW)], qsl[0:64, :],
                                KT[0:64, ds(hb + kc * ACW, ACW)],
                                start=True, stop=True)
                            nc.tensor.matmul(
                                psA2[:, ts(kc, ACW)], qsl[64:128, :],
                                KT[64:128, ds(hb + kc * ACW, ACW)],
                                start=True, stop=True, tile_position=(64, 0))
                        nc.scalar.activation(
                            exp1[:, ds(hb, KHALF)], psA1[:], AF.Exp,
                            scale=scale, accum_out=sp[:, 0 + half : 1 + half])
                        nc.scalar.activation(
                            exp2[:, ds(hb, KHALF)], psA2[:], AF.Exp,
                            scale=scale, accum_out=sp[:, 2 + half : 3 + half])
                    s12 = work.tile([P, 2], F32, tag="s12")
                    nc.vector.tensor_tensor(s12[:, 0:1], sp[:, 0:1], sp[:, 1:2], OP.add)
                    nc.vector.tensor_tensor(s12[:, 1:2], sp[:, 2:3], sp[:, 3:4], OP.add)
                    r12 = work.tile([P, 2], F32, tag="r12")
                    nc.vector.reciprocal(r12[:], s12[:])

                    tmp = work.tile([P, s], BF16, tag="tmp")
                    nc.vector.tensor_scalar(tmp[:], exp2[:], r12[:, 1:2], lam,
                                            OP.mult, OP.mult)
                    comb = work.tile([P, s], BF16, tag="comb")
                    nc.vector.scalar_tensor_tensor(
                        comb[:], exp1[:], r12[:, 0:1], tmp[:], OP.mult, OP.subtract)

                    nc.gpsimd.dma_start(comb_d.ap()[ds(iv * P, P), :], comb[:])
                    nc.sync.dma_start(combbf_d[ds(iv * P, P), :], comb[:])

            # LoopB: per query block: combined^T from DRAM via xbar -> AV
            if "lb" in stage_set:
                with tc.For_i(0, NBLK, 1) as bv:
                    cT4 = work.tile([P, NKT, BW], BF16, tag="cT4")
                    nc.sync.dma_start_transpose(
                        cT4[:], combbf_d[ds(bv * BW, BW), :])
                    psOT = psum.tile([P, BW], F32, tag="ps")
                    for c in range(NKT):
                        nc.tensor.matmul(psOT[:], V[:, c, :], cT4[:, c, :],
                                         start=(c == 0), stop=(c == NKT - 1))
                    ot = work.tile([P, BW], F32, tag="ot")
                    nc.vector.tensor_copy(ot[:], psOT[:])
                    nc.sync.dma_start(out_d.ap()[:, ds(bv * BW, BW)], ot[:])

    nc.compile()
    return nc


def _get_nc(lam: float, n_iter: int = 1):
    key = (round(lam, 6), n_iter)
    if key not in _BUILD_CACHE:
        _BUILD_CACHE[key] = _build(lam, S, D, S // 2, n_iter)
    return _BUILD_CACHE[key]


def _prep_inputs(x, Wq, Wk, Wv):
    bf = ml_dtypes.bfloat16
    x = np.asarray(x, dtype=np.float32)
    wqt = np.ascontiguousarray(np.asarray(Wq, np.float32).T).astype(bf)
    wkt = np.ascontiguousarray(np.asarray(Wk, np.float32).T).astype(bf)
    wvt = np.ascontiguousarray(np.asarray(Wv, np.float32).T).astype(bf)
    roll = np.r_[S // 2 : S, 0 : S // 2]
    in_maps = []
    for c in range(NCORES):
        b, h = divmod(c, 2)
        xb = x[b] if h == 0 else x[b][roll]
        xt = np.ascontiguousarray(xb.T).astype(bf)
        in_maps.append({"xt": xt, "wqt": wqt, "wkt": wkt, "wvt": wvt})
    return in_maps, roll


def kernel(x, Wq, Wk, Wv, lambda_q1, lambda_q2, lambda_k1, lambda_k2):
    from concourse.bass_utils import run_bass_kernel_spmd

    lq1 = np.asarray(lambda_q1, np.float64)
    lq2 = np.asarray(lambda_q2, np.float64)
    lk1 = np.asarray(lambda_k1, np.float64)
    lk2 = np.asarray(lambda_k2, np.float64)
    lam_init = 0.8 - 0.6 * math.exp(-0.3 * DEPTH)
    lam = float(
        np.exp(np.sum(lq1 * lk1)) - np.exp(np.sum(lq2 * lk2)) + lam_init
    )

    nc = _get_nc(lam)
    in_maps, roll = _prep_inputs(x, Wq, Wk, Wv)
    res = run_bass_kernel_spmd(nc, in_maps, core_ids=list(range(NCORES)))

    QH = S // 2
    combined = np.empty((B, S, S), np.float32)
    output = np.empty((B, S, E), np.float32)
    for c in range(NCORES):
        b, h = divmod(c, 2)
        comb = res.results[c]["comb"]
        out = res.results[c]["out"]
        if h == 1:
            comb = comb[:, roll]
        combined[b, h * QH : (h + 1) * QH] = comb
        output[b, h * QH : (h + 1) * QH] = out.T
    return output, combined
